# revision 13
# baseline (speedup 1.0000x reference)
"""Multi-head cross-attention (B=4, S=2048, D=1024, H=16) on 8 Trainium2 cores.

Sharding: hybrid data/tensor parallel. Core c handles batch b = c//2 and
head-group g = c%2 (8 of the 16 heads, i.e. 512 of the 1024 q/k/v dims).
Each core computes a partial out-projection over its 512 attention dims;
the host sums the two partials per batch.

Design (v3):
- ACT engine runs ONLY exp; its ~285us busy is the kernel floor. The
  key-padding mask is applied by zeroing masked keys' V rows and ones
  column (exactly equivalent to -inf logits), so one biasless exp spans
  two key chunks ([128,1024]).
- S=K.T@Q contracts over head_dim=64: issued as PE row-tiled pairs
  (tile_position (0,0)/(64,0)) emitted adjacently so both heads stream
  concurrently on the two array halves.
- Per query block (512 queries) the work is two phases: A = S+exp for
  all 16 key chunks (es kept in a 16-deep SBUF ring), B = the 32 AV
  accumulation matmuls. A(g+1) is emitted interleaved into B(g) so the
  exp stream never waits on AV/normalize; softmax normalize runs off
  the critical path during the next block's A phase.
- All projections (K1-3, Q1-3, V, O) are pumped as small filler batches
  inside A/B so the PE stays dense (HAM stays at 2.4GHz) and projection
  time hides entirely under the exp-bound attention span.
- PSUM: lg_e(2) lg_o(2) av_e(1) av_o(1) pj(2) = 8 banks.

bv is folded into bo on the host (softmax rows sum to 1).
"""

import numpy as np

import concourse.bacc as bacc
import concourse.mybir as mybir
from concourse import tile
from concourse.bass_utils import run_bass_kernel_spmd

F32 = mybir.dt.float32
F16 = mybir.dt.float16
AF = mybir.ActivationFunctionType

B, S, D = 4, 2048, 1024
H, HD = 16, 64
NCORES = 8
NH = 8          # heads per core
OD = NH * HD    # 512 attention dims per core
P = 128
NDC = D // P    # 8 d-chunks
NKC = S // P    # 16 key chunks
NMT = OD // P   # 4 head-pairs
NG = NMT * 4    # 16 query-block groups

_cache = {}


def _build():
    from collections import deque
    from contextlib import ExitStack

    nc = bacc.Bacc(None, target_bir_lowering=False, debug=False)

    x_t = nc.dram_tensor("x_t", [D, S], F16, kind="ExternalInput").ap()
    mem_t = nc.dram_tensor("mem_t", [D, S], F16, kind="ExternalInput").ap()
    wq_t = nc.dram_tensor("wq_t", [D, OD], F16, kind="ExternalInput").ap()
    wk_t = nc.dram_tensor("wk_t", [D, OD], F16, kind="ExternalInput").ap()
    wv_t = nc.dram_tensor("wv_t", [D, OD], F16, kind="ExternalInput").ap()
    wo_t = nc.dram_tensor("wo_t", [OD, D], F16, kind="ExternalInput").ap()
    bq_s = nc.dram_tensor("bq_s", [P, OD // P], F32, kind="ExternalInput").ap()
    bk_s = nc.dram_tensor("bk_s", [P, OD // P], F32, kind="ExternalInput").ap()
    bo_s = nc.dram_tensor("bo_s", [P, D // P], F32, kind="ExternalInput").ap()
    vmask = nc.dram_tensor("vmask", [P, NKC], F32, kind="ExternalInput").ap()
    vmask8 = nc.dram_tensor("vmask8", [P, NKC * NH], F16,
                            kind="ExternalInput").ap()
    out_t = nc.dram_tensor("out_t", [D, S], F32, kind="ExternalOutput").ap()

    x_c = x_t.rearrange("(c p) s -> c p s", p=P)
    m_c = mem_t.rearrange("(c p) s -> c p s", p=P)
    wq_c = wq_t.rearrange("(c p) o -> c p o", p=P)
    wk_c = wk_t.rearrange("(c p) o -> c p o", p=P)
    wv_c = wv_t.rearrange("(c p) o -> c p o", p=P)
    wo_c = wo_t.rearrange("(c p) o -> c p o", p=P)

    with tile.TileContext(nc) as tc, ExitStack() as ctx:
        q_pool = ctx.enter_context(tc.tile_pool(name="qt", bufs=1))
        k_pool = ctx.enter_context(tc.tile_pool(name="kt", bufs=1))
        v_pool = ctx.enter_context(tc.tile_pool(name="va", bufs=1))
        a_pool = ctx.enter_context(tc.tile_pool(name="at", bufs=1))
        c_pool = ctx.enter_context(tc.tile_pool(name="cst", bufs=1))
        w_pool = ctx.enter_context(tc.tile_pool(name="wt", bufs=10))
        e_pool = ctx.enter_context(tc.tile_pool(name="es", bufs=16))
        n_pool = ctx.enter_context(tc.tile_pool(name="nrm", bufs=2))
        o_pool = ctx.enter_context(tc.tile_pool(name="ev", bufs=3))
        psum_pool = ctx.enter_context(tc.tile_pool(name="ps", bufs=1, space="PSUM"))
        m_pool = ctx.enter_context(tc.tile_pool(name="mm", bufs=8))
        x_pool = ctx.enter_context(tc.tile_pool(name="xx", bufs=8))

        # ---- constants ----
        bq_sb = c_pool.tile([P, OD // P], F32, tag="bq")
        bk_sb = c_pool.tile([P, OD // P], F32, tag="bk")
        bo_sb = c_pool.tile([P, D // P], F32, tag="bo")
        vm_sb = c_pool.tile([P, NKC], F32, tag="vm")
        vm8_sb = c_pool.tile([P, NKC, NH], F16, tag="vm8")
        nc.sync.dma_start(out=bq_sb[:], in_=bq_s[:])
        nc.sync.dma_start(out=bk_sb[:], in_=bk_s[:])
        nc.sync.dma_start(out=bo_sb[:], in_=bo_s[:])
        nc.sync.dma_start(out=vm_sb[:], in_=vmask[:])
        nc.sync.dma_start(
            out=vm8_sb[:], in_=vmask8.rearrange("p (s h) -> p s h", h=NH))

        # ---- weight DMAs first (small, needed first), then bulk inputs:
        # memory on both queues (K0/V gate on it), then x (Q0 gates on it)
        w0_tiles = {"k": [], "q": [], "v": []}
        for i in range(NDC):
            wt = w_pool.tile([P, P], F16, tag="w", name="wk0", bufs=10)
            nc.sync.dma_start(out=wt[:], in_=wk_c[i, :, 0:P])
            w0_tiles["k"].append(wt)
        for i in range(NDC):
            wt = w_pool.tile([P, P], F16, tag="wq0", name="wq0", bufs=8)
            nc.gpsimd.dma_start(out=wt[:], in_=wq_c[i, :, 0:P])
            w0_tiles["q"].append(wt)
        for i in range(NDC):
            wt = w_pool.tile([P, OD], F16, tag="wv", name="wvt", bufs=8)
            eng = nc.sync if i % 2 == 0 else nc.gpsimd
            eng.dma_start(out=wt[:], in_=wv_c[i])
            w0_tiles["v"].append(wt)
        # m/x as column blocks [128, 512]: block cb holds tokens/queries
        # [cb*512, (cb+1)*512) so compute can start after ~2MB instead of 8.5
        m_cb = [[None] * NDC for _ in range(4)]
        x_cb = [[None] * NDC for _ in range(4)]
        for cb in range(4):
            for i in range(NDC):
                t = m_pool.tile([P, 512], F16, tag=f"m{cb}", name="mt")
                eng = nc.sync if (i + cb) % 2 == 0 else nc.gpsimd
                eng.dma_start(out=t[:], in_=m_c[i, :, cb * 512:(cb + 1) * 512])
                m_cb[cb][i] = t
            for i in range(NDC):
                t = x_pool.tile([P, 512], F16, tag=f"x{cb}", name="xt")
                eng = nc.sync if (i + cb) % 2 == 1 else nc.gpsimd
                eng.dma_start(out=t[:], in_=x_c[i, :, cb * 512:(cb + 1) * 512])
                x_cb[cb][i] = t

        # ---- persistent tiles ----
        qT = [q_pool.tile([P, S], F16, tag=f"q{m}", name=f"q{m}")
              for m in range(NMT)]
        # kT packs a head pair: partitions 0:64 = head 2m, 64:128 = head 2m+1
        kT = [k_pool.tile([P, S], F16, tag=f"k{m}", name=f"k{m}")
              for m in range(NMT)]
        v_aug = [v_pool.tile([P, NH, 65], F16, tag=f"v{st}", name=f"v{st}")
                 for st in range(NKC)]
        attn = [a_pool.tile([P, S], F16, tag=f"a{m}", name=f"a{m}")
                for m in range(NMT)]

        # ---- filler stream (projections chopped into single-MM steps) ----
        fills = deque()

        def pump(n):
            for _ in range(n):
                if fills:
                    fills.popleft()()

        def kq_proj_steps(wc, src_cb, dst, bias, m, w_tiles=None,
                          halves=range(4)):
            if w_tiles is None:
                w_tiles = []

                def load_w():
                    for i in range(NDC):
                        wt = w_pool.tile([P, P], F16, tag="w", name="wkq",
                                         bufs=10)
                        nc.sync.dma_start(
                            out=wt[:], in_=wc[i, :, m * P:(m + 1) * P])
                        w_tiles.append(wt)
                steps = [load_w]
            else:
                steps = []
            for half in halves:
                csl = slice(half * 512, (half + 1) * 512)
                ps = []

                def mm(i, ps=ps, half=half):
                    if i == 0:
                        ps.append(psum_pool.tile([P, 512], F32, tag="pj",
                                                 name="pskq", bufs=2))
                    nc.tensor.matmul(
                        ps[0][:], w_tiles[i][:], src_cb[half][i][:],
                        start=(i == 0), stop=(i == NDC - 1))
                for i in range(NDC):
                    steps.append(lambda i=i, mm=mm: mm(i))

                def evac(ps=ps, csl=csl):
                    nc.vector.tensor_scalar_add(
                        dst[:, csl], ps[0][:], bias[:, m:m + 1])
                steps.append(evac)
            return steps

        # V-proj: per-token-chunk projection (preloaded weights)
        wv_tiles = w0_tiles["v"]

        def v_proj(st):
            ps = psum_pool.tile([P, 512], F32, tag="pj", name="psv", bufs=2)
            for i in range(NDC):
                nc.tensor.matmul(
                    ps[:], m_cb[st // 4][i][:, (st % 4) * P:(st % 4 + 1) * P],
                    wv_tiles[i][:],
                    start=(i == 0), stop=(i == NDC - 1))
            nc.vector.tensor_scalar_mul(
                v_aug[st][:, 0:NH, 0:64],
                ps[:].rearrange("p (h d) -> p h d", h=NH),
                vm_sb[:, st:st + 1])
            nc.gpsimd.tensor_copy(
                v_aug[st][:, 0:NH, 64:65], vm8_sb[:, st, :].unsqueeze(2))

        wo_tiles = [[None] * NMT for _ in range(D // P)]

        def load_wo(m):
            for i in range(NMT):
                wt = w_pool.tile([P, P], F16, tag="wo", name="wot", bufs=32)
                nc.sync.dma_start(out=wt[:], in_=wo_c[i, :, m * P:(m + 1) * P])
                wo_tiles[m][i] = wt

        def o_proj_steps(m, jb):
            ps = []

            def mm(i):
                if i == 0:
                    ps.append(psum_pool.tile([P, 512], F32, tag="pj",
                                             name="pso", bufs=2))
                nc.tensor.matmul(
                    ps[0][:], wo_tiles[m][i][:],
                    attn[i][:, jb * 512:(jb + 1) * 512],
                    start=(i == 0), stop=(i == NMT - 1))
            steps = [lambda i=i, mm=mm: mm(i) for i in range(NMT)]

            def evac():
                ev = o_pool.tile([P, 512], F32, tag="ev")
                nc.vector.tensor_scalar_add(ev[:], ps[0][:], bo_sb[:, m:m + 1])
                nc.sync.dma_start(
                    out=out_t[m * P:(m + 1) * P, jb * 512:(jb + 1) * 512],
                    in_=ev[:])
            steps.append(evac)
            return steps

        # queue the hidden projections (emitted later via pump/loop-top)
        q0_rest = [kq_proj_steps(wq_c, x_cb, qT[0], bq_sb, 0,
                                 w_tiles=w0_tiles["q"], halves=[h])
                   for h in (1, 2, 3)]
        for m in (1, 2, 3):
            fills.extend(kq_proj_steps(wk_c, m_cb, kT[m], bk_sb, m))
            fills.extend(kq_proj_steps(wq_c, x_cb, qT[m], bq_sb, m))

        # ---- attention pipeline ----
        esbuf = {}
        avbuf = {}

        def emit_A(g, k2):
            mt, qb = divmod(g, 4)
            qsl = slice(qb * 512, (qb + 1) * 512)
            ka, kb = 2 * k2, 2 * k2 + 1
            lg_e = psum_pool.tile([P, 1024], F32, tag="lg", name="lg_e",
                                  bufs=2)
            lg_o = psum_pool.tile([P, 1024], F32, tag="lg", name="lg_o",
                                  bufs=2)
            for half, kc in ((0, ka), (1, kb)):
                nc.tensor.matmul(
                    lg_e[:, half * 512:(half + 1) * 512],
                    kT[mt][0:64, kc * P:(kc + 1) * P],
                    qT[mt][0:64, qsl], start=True, stop=True)
                nc.tensor.matmul(
                    lg_o[:, half * 512:(half + 1) * 512],
                    kT[mt][64:128, kc * P:(kc + 1) * P],
                    qT[mt][64:128, qsl], start=True, stop=True)
            es_e = e_pool.tile([P, 1024], F16, tag="es", bufs=16)
            nc.scalar.activation(es_e[:], lg_e[:], AF.Exp, scale=0.125)
            es_o = e_pool.tile([P, 1024], F16, tag="es", bufs=16)
            nc.scalar.activation(es_o[:], lg_o[:], AF.Exp, scale=0.125)
            esbuf[(g, k2)] = (es_e, es_o)
            if 2 <= g < 4:
                pump(4)
            elif g >= 4:
                pump(3)

        def emit_B(g, k2):
            mt, qb = divmod(g, 4)
            he, ho = 2 * mt, 2 * mt + 1
            if k2 == 0:
                avbuf[g] = (
                    psum_pool.tile([P, 512], F32, tag="av_e", name="av_e"),
                    psum_pool.tile([P, 512], F32, tag="av_o", name="av_o"))
            av_e, av_o = avbuf[g]
            if g == 0 and k2 < 6:
                v_proj(4 + 2 * k2)
                v_proj(5 + 2 * k2)
            if g == 1 and k2 < 4:
                pass
            es_e, es_o = esbuf.pop((g, k2))
            ka, kb = 2 * k2, 2 * k2 + 1
            for half, kc in ((0, ka), (1, kb)):
                va = v_aug[kc][:].rearrange("p h d -> p (h d)")
                nc.tensor.matmul(
                    av_e[0:65, :], va[:, 65 * he:65 * he + 65],
                    es_e[:, half * 512:(half + 1) * 512],
                    start=(k2 == 0 and half == 0),
                    stop=(k2 == NKC // 2 - 1 and half == 1))
            for half, kc in ((0, ka), (1, kb)):
                va = v_aug[kc][:].rearrange("p h d -> p (h d)")
                nc.tensor.matmul(
                    av_o[0:65, :], va[:, 65 * ho:65 * ho + 65],
                    es_o[:, half * 512:(half + 1) * 512],
                    start=(k2 == 0 and half == 0),
                    stop=(k2 == NKC // 2 - 1 and half == 1))
            if g >= 1:
                pump(2)

        def emit_norm(g):
            mt, qb = divmod(g, 4)
            qsl = slice(qb * 512, (qb + 1) * 512)
            av_e, av_o = avbuf.pop(g)
            for ro, av in ((0, av_e), (64, av_o)):
                dn = n_pool.tile([1, 512], F32, tag="dn")
                r0 = n_pool.tile([1, 512], F32, tag="r0")
                bc = n_pool.tile([64, 512], F32, tag="bc")
                nc.vector.tensor_copy(dn[:], av[64:65, :])
                nc.vector.reciprocal_approx_fast(out=r0[:], in_=dn[:])
                nc.gpsimd.partition_broadcast(bc[:], r0[:])
                nc.vector.tensor_mul(
                    attn[mt][ro:ro + 64, qsl], av[0:64, :], bc[:])

        # staged startup: K0 half-by-half interleaved with A(0) so the
        # first exp fires as soon as 2MB (m/x column-block 0) has landed
        k0 = kq_proj_steps(wk_c, m_cb, kT[0], bk_sb, 0, w_tiles=w0_tiles["k"])
        q0 = kq_proj_steps(wq_c, x_cb, qT[0], bq_sb, 0, w_tiles=w0_tiles["q"],
                           halves=[0])
        for step in k0[0:9]:     # K0 half0
            step()
        for step in q0:          # Q0 half0 (all A(0) needs)
            step()
        emit_A(0, 0)
        emit_A(0, 1)
        for step in k0[9:18]:    # K0 half1
            step()
        v_proj(0)
        emit_A(0, 2)
        emit_A(0, 3)
        for step in k0[18:27]:   # K0 half2
            step()
        v_proj(1)
        emit_A(0, 4)
        emit_A(0, 5)
        for step in k0[27:36]:   # K0 half3
            step()
        v_proj(2)
        v_proj(3)
        emit_A(0, 6)
        emit_A(0, 7)
        for g in range(NG):
            if g < 3:            # Q0 half g+1 gates A(g+1) emission
                for step in q0_rest[g]:
                    step()
            for k2 in range(NKC // 2):
                emit_B(g, k2)
                if g + 1 < NG:
                    emit_A(g + 1, k2)
            emit_norm(g)
            # register O-projection for completed query columns
            mt, qb = divmod(g, 4)
            if mt == NMT - 1 and qb < 3:
                for m in range(D // P):
                    if qb == 0:
                        fills.append(lambda m=m: load_wo(m))
                    fills.extend(o_proj_steps(m, qb))

        # ---- drain fills, then O-proj for the last column block ----
        while fills:
            fills.popleft()()
        for m in range(D // P):
            for step in o_proj_steps(m, 3):
                step()

    nc.compile()
    return nc


def _prep_inputs(x, memory, mask, wq, bq, wk, bk, wv, bv, wo, bo):
    f = np.float32
    h = np.float16
    wqT = np.ascontiguousarray(wq.T, dtype=f)
    wkT = np.ascontiguousarray(wk.T, dtype=f)
    wvT = np.ascontiguousarray(wv.T, dtype=f)
    woT = np.ascontiguousarray(wo.T, dtype=f)
    bo_eff = (bo.astype(f) + wo.astype(f) @ bv.astype(f))
    zeros_bo = np.zeros_like(bo_eff)
    in_maps = []
    for c in range(NCORES):
        b, g = divmod(c, 2)
        sl = slice(g * OD, (g + 1) * OD)
        bo_c = bo_eff if g == 0 else zeros_bo
        vm = np.where(mask[b], np.float32(0.0), np.float32(1.0)).astype(f)
        vm_s = np.ascontiguousarray(vm.reshape(NKC, P).T)      # [P, NKC]
        vm8 = np.repeat(vm_s.astype(h)[:, :, None], NH, axis=2)  # [P,NKC,NH]
        in_maps.append({
            "x_t": np.ascontiguousarray(x[b].T, dtype=h),
            "mem_t": np.ascontiguousarray(memory[b].T, dtype=h),
            "wq_t": np.ascontiguousarray(wqT[:, sl]).astype(h),
            "wk_t": np.ascontiguousarray(wkT[:, sl]).astype(h),
            "wv_t": np.ascontiguousarray(wvT[:, sl]).astype(h),
            "wo_t": np.ascontiguousarray(woT[sl, :]).astype(h),
            "bq_s": np.ascontiguousarray(bq[sl].astype(f).reshape(OD // P, P).T),
            "bk_s": np.ascontiguousarray(bk[sl].astype(f).reshape(OD // P, P).T),
            "bo_s": np.ascontiguousarray(bo_c.reshape(D // P, P).T),
            "vmask": vm_s,
            "vmask8": np.ascontiguousarray(vm8.reshape(P, NKC * NH)),
        })
    return in_maps


def kernel(x, memory, mask, wq, bq, wk, bk, wv, bv, wo, bo, **run_kwargs):
    x = np.asarray(x, dtype=np.float32)
    memory = np.asarray(memory, dtype=np.float32)
    mask = np.asarray(mask)
    if "nc" not in _cache:
        _cache["nc"] = _build()
    nc = _cache["nc"]
    in_maps = _prep_inputs(x, memory, mask, wq, bq, wk, bk, wv, bv, wo, bo)
    res = run_bass_kernel_spmd(nc, in_maps, list(range(NCORES)), **run_kwargs)
    out = np.empty((B, S, D), dtype=np.float32)
    for b in range(B):
        part = res.results[2 * b]["out_t"] + res.results[2 * b + 1]["out_t"]
        out[b] = part.T
    if run_kwargs:
        _cache["last_results"] = res
    return out


# revision 16
# speedup vs baseline: 1.0724x; 1.0724x over previous
"""Multi-head cross-attention (B=4, S=2048, D=1024, H=16) on 8 Trainium2 cores.

Sharding: hybrid data/tensor parallel. Core c handles batch b = c//2 and
head-group g = c%2 (8 of the 16 heads, i.e. 512 of the 1024 q/k/v dims).
Each core computes a partial out-projection over its 512 attention dims;
the host sums the two partials per batch.

Design (v3):
- ACT engine runs ONLY exp; its ~285us busy is the kernel floor. The
  key-padding mask is applied by zeroing masked keys' V rows and ones
  column (exactly equivalent to -inf logits), so one biasless exp spans
  two key chunks ([128,1024]).
- S=K.T@Q contracts over head_dim=64: issued as PE row-tiled pairs
  (tile_position (0,0)/(64,0)) emitted adjacently so both heads stream
  concurrently on the two array halves.
- Per query block (512 queries) the work is two phases: A = S+exp for
  all 16 key chunks (es kept in a 16-deep SBUF ring), B = the 32 AV
  accumulation matmuls. A(g+1) is emitted interleaved into B(g) so the
  exp stream never waits on AV/normalize; softmax normalize runs off
  the critical path during the next block's A phase.
- All projections (K1-3, Q1-3, V, O) are pumped as small filler batches
  inside A/B so the PE stays dense (HAM stays at 2.4GHz) and projection
  time hides entirely under the exp-bound attention span.
- PSUM: lg_e(2) lg_o(2) av_e(1) av_o(1) pj(2) = 8 banks.

bv is folded into bo on the host (softmax rows sum to 1).
"""

import numpy as np

import concourse.bacc as bacc
import concourse.mybir as mybir
from concourse import tile
from concourse.bass_utils import run_bass_kernel_spmd

F32 = mybir.dt.float32
F16 = mybir.dt.float16
AF = mybir.ActivationFunctionType

B, S, D = 4, 2048, 1024
H, HD = 16, 64
NCORES = 8
NH = 8          # heads per core
OD = NH * HD    # 512 attention dims per core
P = 128
NDC = D // P    # 8 d-chunks
NKC = S // P    # 16 key chunks
NMT = OD // P   # 4 head-pairs
NG = NMT * 4    # 16 query-block groups

_cache = {}


def _build():
    from collections import deque
    from contextlib import ExitStack

    nc = bacc.Bacc(None, target_bir_lowering=False, debug=False)

    x_t = nc.dram_tensor("x_t", [D, S], F16, kind="ExternalInput").ap()
    mem_t = nc.dram_tensor("mem_t", [D, S], F16, kind="ExternalInput").ap()
    wq_t = nc.dram_tensor("wq_t", [D, OD], F16, kind="ExternalInput").ap()
    wk_t = nc.dram_tensor("wk_t", [D, OD], F16, kind="ExternalInput").ap()
    wv_t = nc.dram_tensor("wv_t", [D, OD], F16, kind="ExternalInput").ap()
    wo_t = nc.dram_tensor("wo_t", [OD, D], F16, kind="ExternalInput").ap()
    bq_s = nc.dram_tensor("bq_s", [P, OD // P], F32, kind="ExternalInput").ap()
    bk_s = nc.dram_tensor("bk_s", [P, OD // P], F32, kind="ExternalInput").ap()
    bo_s = nc.dram_tensor("bo_s", [P, D // P], F32, kind="ExternalInput").ap()
    vmask = nc.dram_tensor("vmask", [P, NKC], F32, kind="ExternalInput").ap()
    vmask8 = nc.dram_tensor("vmask8", [P, NKC * NH], F16,
                            kind="ExternalInput").ap()
    out_t = nc.dram_tensor("out_t", [D, S], F32, kind="ExternalOutput").ap()

    x_c = x_t.rearrange("(c p) s -> c p s", p=P)
    m_c = mem_t.rearrange("(c p) s -> c p s", p=P)
    wq_c = wq_t.rearrange("(c p) o -> c p o", p=P)
    wk_c = wk_t.rearrange("(c p) o -> c p o", p=P)
    wv_c = wv_t.rearrange("(c p) o -> c p o", p=P)
    wo_c = wo_t.rearrange("(c p) o -> c p o", p=P)

    with tile.TileContext(nc) as tc, ExitStack() as ctx:
        q_pool = ctx.enter_context(tc.tile_pool(name="qt", bufs=1))
        k_pool = ctx.enter_context(tc.tile_pool(name="kt", bufs=1))
        v_pool = ctx.enter_context(tc.tile_pool(name="va", bufs=1))
        a_pool = ctx.enter_context(tc.tile_pool(name="at", bufs=1))
        c_pool = ctx.enter_context(tc.tile_pool(name="cst", bufs=1))
        w_pool = ctx.enter_context(tc.tile_pool(name="wt", bufs=10))
        e_pool = ctx.enter_context(tc.tile_pool(name="es", bufs=16))
        n_pool = ctx.enter_context(tc.tile_pool(name="nrm", bufs=2))
        o_pool = ctx.enter_context(tc.tile_pool(name="ev", bufs=3))
        psum_pool = ctx.enter_context(tc.tile_pool(name="ps", bufs=1, space="PSUM"))
        m_pool = ctx.enter_context(tc.tile_pool(name="mm", bufs=8))
        x_pool = ctx.enter_context(tc.tile_pool(name="xx", bufs=8))

        # ---- constants ----
        bq_sb = c_pool.tile([P, OD // P], F32, tag="bq")
        bk_sb = c_pool.tile([P, OD // P], F32, tag="bk")
        bo_sb = c_pool.tile([P, D // P], F32, tag="bo")
        vm_sb = c_pool.tile([P, NKC], F32, tag="vm")
        vm8_sb = c_pool.tile([P, NKC, NH], F16, tag="vm8")
        nc.sync.dma_start(out=bq_sb[:], in_=bq_s[:])
        nc.sync.dma_start(out=bk_sb[:], in_=bk_s[:])
        nc.sync.dma_start(out=bo_sb[:], in_=bo_s[:])
        nc.sync.dma_start(out=vm_sb[:], in_=vmask[:])
        nc.sync.dma_start(
            out=vm8_sb[:], in_=vmask8.rearrange("p (s h) -> p s h", h=NH))

        # ---- weight DMAs first (small, needed first), then bulk inputs:
        # memory on both queues (K0/V gate on it), then x (Q0 gates on it)
        w0_tiles = {"k": [], "q": [], "v": []}
        for i in range(NDC):
            wt = w_pool.tile([P, P], F16, tag="w", name="wk0", bufs=10)
            nc.sync.dma_start(out=wt[:], in_=wk_c[i, :, 0:P])
            w0_tiles["k"].append(wt)
        for i in range(NDC):
            wt = w_pool.tile([P, P], F16, tag="wq0", name="wq0", bufs=8)
            nc.gpsimd.dma_start(out=wt[:], in_=wq_c[i, :, 0:P])
            w0_tiles["q"].append(wt)
        for i in range(NDC):
            wt = w_pool.tile([P, OD], F16, tag="wv", name="wvt", bufs=8)
            eng = nc.sync if i % 2 == 0 else nc.gpsimd
            eng.dma_start(out=wt[:], in_=wv_c[i])
            w0_tiles["v"].append(wt)
        # m/x as column blocks [128, 512]: block cb holds tokens/queries
        # [cb*512, (cb+1)*512) so compute can start after ~2MB instead of 8.5
        m_cb = [[None] * NDC for _ in range(4)]
        x_cb = [[None] * NDC for _ in range(4)]
        for cb in range(4):
            for i in range(NDC):
                t = m_pool.tile([P, 512], F16, tag=f"m{cb}", name="mt")
                eng = nc.sync if (i + cb) % 2 == 0 else nc.gpsimd
                eng.dma_start(out=t[:], in_=m_c[i, :, cb * 512:(cb + 1) * 512])
                m_cb[cb][i] = t
            for i in range(NDC):
                t = x_pool.tile([P, 512], F16, tag=f"x{cb}", name="xt")
                eng = nc.sync if (i + cb) % 2 == 1 else nc.gpsimd
                eng.dma_start(out=t[:], in_=x_c[i, :, cb * 512:(cb + 1) * 512])
                x_cb[cb][i] = t

        # ---- persistent tiles ----
        qT = [q_pool.tile([P, S], F16, tag=f"q{m}", name=f"q{m}")
              for m in range(NMT)]
        # kT packs a head pair: partitions 0:64 = head 2m, 64:128 = head 2m+1
        kT = [k_pool.tile([P, S], F16, tag=f"k{m}", name=f"k{m}")
              for m in range(NMT)]
        v_aug = [v_pool.tile([P, NH, 65], F16, tag=f"v{st}", name=f"v{st}")
                 for st in range(NKC)]
        attn = [a_pool.tile([P, S], F16, tag=f"a{m}", name=f"a{m}")
                for m in range(NMT)]

        # ---- filler stream (projections chopped into single-MM steps) ----
        fills = deque()

        def pump(n):
            for _ in range(n):
                if fills:
                    fills.popleft()()

        def kq_proj_steps(wc, src_cb, dst, bias, m, w_tiles=None,
                          halves=range(4)):
            if w_tiles is None:
                w_tiles = []

                def load_w():
                    for i in range(NDC):
                        wt = w_pool.tile([P, P], F16, tag="w", name="wkq",
                                         bufs=10)
                        nc.sync.dma_start(
                            out=wt[:], in_=wc[i, :, m * P:(m + 1) * P])
                        w_tiles.append(wt)
                steps = [load_w]
            else:
                steps = []
            for half in halves:
                csl = slice(half * 512, (half + 1) * 512)
                ps = []

                def mm(i, ps=ps, half=half):
                    if i == 0:
                        ps.append(psum_pool.tile([P, 512], F32, tag="pj",
                                                 name="pskq", bufs=2))
                    nc.tensor.matmul(
                        ps[0][:], w_tiles[i][:], src_cb[half][i][:],
                        start=(i == 0), stop=(i == NDC - 1))
                for i in range(NDC):
                    steps.append(lambda i=i, mm=mm: mm(i))

                def evac(ps=ps, csl=csl):
                    nc.vector.tensor_scalar_add(
                        dst[:, csl], ps[0][:], bias[:, m:m + 1])
                steps.append(evac)
            return steps

        # V-proj: per-token-chunk projection (preloaded weights)
        wv_tiles = w0_tiles["v"]

        def v_proj(st):
            ps = psum_pool.tile([P, 512], F32, tag="pj", name="psv", bufs=2)
            for i in range(NDC):
                nc.tensor.matmul(
                    ps[:], m_cb[st // 4][i][:, (st % 4) * P:(st % 4 + 1) * P],
                    wv_tiles[i][:],
                    start=(i == 0), stop=(i == NDC - 1))
            nc.vector.tensor_scalar_mul(
                v_aug[st][:, 0:NH, 0:64],
                ps[:].rearrange("p (h d) -> p h d", h=NH),
                vm_sb[:, st:st + 1])
            nc.gpsimd.tensor_copy(
                v_aug[st][:, 0:NH, 64:65], vm8_sb[:, st, :].unsqueeze(2))

        wo_tiles = [[None] * NMT for _ in range(D // P)]

        def load_wo(m):
            for i in range(NMT):
                wt = w_pool.tile([P, P], F16, tag="wo", name="wot", bufs=32)
                nc.sync.dma_start(out=wt[:], in_=wo_c[i, :, m * P:(m + 1) * P])
                wo_tiles[m][i] = wt

        def o_proj_steps(m, jb):
            ps = []

            def mm(i):
                if i == 0:
                    ps.append(psum_pool.tile([P, 512], F32, tag="pj",
                                             name="pso", bufs=2))
                nc.tensor.matmul(
                    ps[0][:], wo_tiles[m][i][:],
                    attn[i][:, jb * 512:(jb + 1) * 512],
                    start=(i == 0), stop=(i == NMT - 1))
            steps = [lambda i=i, mm=mm: mm(i) for i in range(NMT)]

            def evac():
                ev = o_pool.tile([P, 512], F32, tag="ev")
                nc.vector.tensor_scalar_add(ev[:], ps[0][:], bo_sb[:, m:m + 1])
                nc.sync.dma_start(
                    out=out_t[m * P:(m + 1) * P, jb * 512:(jb + 1) * 512],
                    in_=ev[:])
            steps.append(evac)
            return steps

        # Q weights for all head-pairs preloaded (tiny); halves emitted
        # on demand at loop tops
        wq_all = [w0_tiles["q"]]
        for m in (1, 2, 3):
            tiles = []
            for i in range(NDC):
                wt = w_pool.tile([P, P], F16, tag=f"wq{m}", name="wqm", bufs=8)
                nc.sync.dma_start(out=wt[:], in_=wq_c[i, :, m * P:(m + 1) * P])
                tiles.append(wt)
            wq_all.append(tiles)

        # ---- attention pipeline ----
        esbuf = {}
        avbuf = {}

        def emit_A(g, k2):
            qb, mt = divmod(g, 4)
            qsl = slice(qb * 512, (qb + 1) * 512)
            ka, kb = 2 * k2, 2 * k2 + 1
            lg_e = psum_pool.tile([P, 1024], F32, tag="lg", name="lg_e",
                                  bufs=2)
            lg_o = psum_pool.tile([P, 1024], F32, tag="lg", name="lg_o",
                                  bufs=2)
            for half, kc in ((0, ka), (1, kb)):
                nc.tensor.matmul(
                    lg_e[:, half * 512:(half + 1) * 512],
                    kT[mt][0:64, kc * P:(kc + 1) * P],
                    qT[mt][0:64, qsl], start=True, stop=True)
                nc.tensor.matmul(
                    lg_o[:, half * 512:(half + 1) * 512],
                    kT[mt][64:128, kc * P:(kc + 1) * P],
                    qT[mt][64:128, qsl], start=True, stop=True)
            es_e = e_pool.tile([P, 1024], F16, tag="es", bufs=16)
            nc.scalar.activation(es_e[:], lg_e[:], AF.Exp, scale=0.125)
            es_o = e_pool.tile([P, 1024], F16, tag="es", bufs=16)
            nc.scalar.activation(es_o[:], lg_o[:], AF.Exp, scale=0.125)
            esbuf[(g, k2)] = (es_e, es_o)
            if 2 <= g < 4:
                pump(4)
            elif g >= 4:
                pump(3)

        def emit_B(g, k2):
            qb, mt = divmod(g, 4)
            he, ho = 2 * mt, 2 * mt + 1
            if k2 == 0:
                avbuf[g] = (
                    psum_pool.tile([P, 512], F32, tag="av_e", name="av_e"),
                    psum_pool.tile([P, 512], F32, tag="av_o", name="av_o"))
            av_e, av_o = avbuf[g]
            if g == 0 and k2 < 4:
                v_proj(8 + 2 * k2)
                v_proj(9 + 2 * k2)
            es_e, es_o = esbuf.pop((g, k2))
            ka, kb = 2 * k2, 2 * k2 + 1
            for half, kc in ((0, ka), (1, kb)):
                va = v_aug[kc][:].rearrange("p h d -> p (h d)")
                nc.tensor.matmul(
                    av_e[0:65, :], va[:, 65 * he:65 * he + 65],
                    es_e[:, half * 512:(half + 1) * 512],
                    start=(k2 == 0 and half == 0),
                    stop=(k2 == NKC // 2 - 1 and half == 1))
            for half, kc in ((0, ka), (1, kb)):
                va = v_aug[kc][:].rearrange("p h d -> p (h d)")
                nc.tensor.matmul(
                    av_o[0:65, :], va[:, 65 * ho:65 * ho + 65],
                    es_o[:, half * 512:(half + 1) * 512],
                    start=(k2 == 0 and half == 0),
                    stop=(k2 == NKC // 2 - 1 and half == 1))
            if g >= 1:
                pump(2)

        def emit_norm(g):
            qb, mt = divmod(g, 4)
            qsl = slice(qb * 512, (qb + 1) * 512)
            av_e, av_o = avbuf.pop(g)
            for ro, av in ((0, av_e), (64, av_o)):
                dn = n_pool.tile([1, 512], F32, tag="dn")
                r0 = n_pool.tile([1, 512], F32, tag="r0")
                bc = n_pool.tile([64, 512], F32, tag="bc")
                nc.vector.tensor_copy(dn[:], av[64:65, :])
                nc.vector.reciprocal_approx_fast(out=r0[:], in_=dn[:])
                nc.gpsimd.partition_broadcast(bc[:], r0[:])
                nc.vector.tensor_mul(
                    attn[mt][ro:ro + 64, qsl], av[0:64, :], bc[:])

        # staged startup: K0 half-by-half interleaved with A(0) so the
        # first exp fires as soon as 2MB (m/x column-block 0) has landed
        k0 = kq_proj_steps(wk_c, m_cb, kT[0], bk_sb, 0, w_tiles=w0_tiles["k"])
        q0 = kq_proj_steps(wq_c, x_cb, qT[0], bq_sb, 0, w_tiles=w0_tiles["q"],
                           halves=[0])
        for step in k0[0:9]:     # K0 half0
            step()
        for step in q0:          # Q0 half0 (all A(0) needs)
            step()
        emit_A(0, 0)
        emit_A(0, 1)
        for step in k0[9:18]:    # K0 half1
            step()
        v_proj(0)
        v_proj(1)
        emit_A(0, 2)
        emit_A(0, 3)
        for step in k0[18:27]:   # K0 half2
            step()
        v_proj(2)
        v_proj(3)
        emit_A(0, 4)
        emit_A(0, 5)
        for step in k0[27:36]:   # K0 half3
            step()
        v_proj(4)
        v_proj(5)
        emit_A(0, 6)
        emit_A(0, 7)
        v_proj(6)
        v_proj(7)
        # K1-3 run inside the DMA-bound startup window (PE otherwise idle)
        for m in (1, 2, 3):
            for step in kq_proj_steps(wk_c, m_cb, kT[m], bk_sb, m):
                step()
        # qb-major: g = (qb, mt); every 4 g's one full query column finishes,
        # so O-proj + out DMA stream from g=4 onward instead of at the tail
        qh_done = {(0, 0)}
        for g in range(NG):
            if g + 1 < NG:
                qb1, mt1 = divmod(g + 1, 4)
                if (mt1, qb1) not in qh_done:   # Q_{mt1} half qb1
                    qh_done.add((mt1, qb1))
                    for step in kq_proj_steps(
                            wq_c, x_cb, qT[mt1], bq_sb, mt1,
                            w_tiles=wq_all[mt1], halves=[qb1]):
                        step()
            for k2 in range(NKC // 2):
                emit_B(g, k2)
                if g + 1 < NG:
                    emit_A(g + 1, k2)
            emit_norm(g)
            qb, mt = divmod(g, 4)
            if mt == NMT - 1 and qb < 3:
                for m in range(D // P):
                    if qb == 0:
                        fills.append(lambda m=m: load_wo(m))
                    fills.extend(o_proj_steps(m, qb))

        # ---- drain fills, then O-proj for the last column block ----
        while fills:
            fills.popleft()()
        for m in range(D // P):
            for step in o_proj_steps(m, 3):
                step()

    nc.compile()
    return nc


def _prep_inputs(x, memory, mask, wq, bq, wk, bk, wv, bv, wo, bo):
    f = np.float32
    h = np.float16
    wqT = np.ascontiguousarray(wq.T, dtype=f)
    wkT = np.ascontiguousarray(wk.T, dtype=f)
    wvT = np.ascontiguousarray(wv.T, dtype=f)
    woT = np.ascontiguousarray(wo.T, dtype=f)
    bo_eff = (bo.astype(f) + wo.astype(f) @ bv.astype(f))
    zeros_bo = np.zeros_like(bo_eff)
    in_maps = []
    for c in range(NCORES):
        b, g = divmod(c, 2)
        sl = slice(g * OD, (g + 1) * OD)
        bo_c = bo_eff if g == 0 else zeros_bo
        vm = np.where(mask[b], np.float32(0.0), np.float32(1.0)).astype(f)
        vm_s = np.ascontiguousarray(vm.reshape(NKC, P).T)      # [P, NKC]
        vm8 = np.repeat(vm_s.astype(h)[:, :, None], NH, axis=2)  # [P,NKC,NH]
        in_maps.append({
            "x_t": np.ascontiguousarray(x[b].T, dtype=h),
            "mem_t": np.ascontiguousarray(memory[b].T, dtype=h),
            "wq_t": np.ascontiguousarray(wqT[:, sl]).astype(h),
            "wk_t": np.ascontiguousarray(wkT[:, sl]).astype(h),
            "wv_t": np.ascontiguousarray(wvT[:, sl]).astype(h),
            "wo_t": np.ascontiguousarray(woT[sl, :]).astype(h),
            "bq_s": np.ascontiguousarray(bq[sl].astype(f).reshape(OD // P, P).T),
            "bk_s": np.ascontiguousarray(bk[sl].astype(f).reshape(OD // P, P).T),
            "bo_s": np.ascontiguousarray(bo_c.reshape(D // P, P).T),
            "vmask": vm_s,
            "vmask8": np.ascontiguousarray(vm8.reshape(P, NKC * NH)),
        })
    return in_maps


def kernel(x, memory, mask, wq, bq, wk, bk, wv, bv, wo, bo, **run_kwargs):
    x = np.asarray(x, dtype=np.float32)
    memory = np.asarray(memory, dtype=np.float32)
    mask = np.asarray(mask)
    if "nc" not in _cache:
        _cache["nc"] = _build()
    nc = _cache["nc"]
    in_maps = _prep_inputs(x, memory, mask, wq, bq, wk, bk, wv, bv, wo, bo)
    res = run_bass_kernel_spmd(nc, in_maps, list(range(NCORES)), **run_kwargs)
    out = np.empty((B, S, D), dtype=np.float32)
    for b in range(B):
        part = res.results[2 * b]["out_t"] + res.results[2 * b + 1]["out_t"]
        out[b] = part.T
    if run_kwargs:
        _cache["last_results"] = res
    return out


# revision 17
# speedup vs baseline: 1.0793x; 1.0064x over previous
"""Multi-head cross-attention (B=4, S=2048, D=1024, H=16) on 8 Trainium2 cores.

Sharding: hybrid data/tensor parallel. Core c handles batch b = c//2 and
head-group g = c%2 (8 of the 16 heads, i.e. 512 of the 1024 q/k/v dims).
Each core computes a partial out-projection over its 512 attention dims;
the host sums the two partials per batch.

Design (v3):
- ACT engine runs ONLY exp; its ~285us busy is the kernel floor. The
  key-padding mask is applied by zeroing masked keys' V rows and ones
  column (exactly equivalent to -inf logits), so one biasless exp spans
  two key chunks ([128,1024]).
- S=K.T@Q contracts over head_dim=64: issued as PE row-tiled pairs
  (tile_position (0,0)/(64,0)) emitted adjacently so both heads stream
  concurrently on the two array halves.
- Per query block (512 queries) the work is two phases: A = S+exp for
  all 16 key chunks (es kept in a 16-deep SBUF ring), B = the 32 AV
  accumulation matmuls. A(g+1) is emitted interleaved into B(g) so the
  exp stream never waits on AV/normalize; softmax normalize runs off
  the critical path during the next block's A phase.
- All projections (K1-3, Q1-3, V, O) are pumped as small filler batches
  inside A/B so the PE stays dense (HAM stays at 2.4GHz) and projection
  time hides entirely under the exp-bound attention span.
- PSUM: lg_e(2) lg_o(2) av_e(1) av_o(1) pj(2) = 8 banks.

bv is folded into bo on the host (softmax rows sum to 1).
"""

import numpy as np

import concourse.bacc as bacc
import concourse.mybir as mybir
from concourse import tile
from concourse.bass_utils import run_bass_kernel_spmd

F32 = mybir.dt.float32
F16 = mybir.dt.float16
AF = mybir.ActivationFunctionType

B, S, D = 4, 2048, 1024
H, HD = 16, 64
NCORES = 8
NH = 8          # heads per core
OD = NH * HD    # 512 attention dims per core
P = 128
NDC = D // P    # 8 d-chunks
NKC = S // P    # 16 key chunks
NMT = OD // P   # 4 head-pairs
NG = NMT * 4    # 16 query-block groups

_cache = {}


def _build():
    from collections import deque
    from contextlib import ExitStack

    nc = bacc.Bacc(None, target_bir_lowering=False, debug=False)

    x_t = nc.dram_tensor("x_t", [D, S], F16, kind="ExternalInput").ap()
    mem_t = nc.dram_tensor("mem_t", [D, S], F16, kind="ExternalInput").ap()
    wq_t = nc.dram_tensor("wq_t", [D, OD], F16, kind="ExternalInput").ap()
    wk_t = nc.dram_tensor("wk_t", [D, OD], F16, kind="ExternalInput").ap()
    wv_t = nc.dram_tensor("wv_t", [D, OD], F16, kind="ExternalInput").ap()
    wo_t = nc.dram_tensor("wo_t", [OD, D], F16, kind="ExternalInput").ap()
    bq_s = nc.dram_tensor("bq_s", [P, OD // P], F32, kind="ExternalInput").ap()
    bk_s = nc.dram_tensor("bk_s", [P, OD // P], F32, kind="ExternalInput").ap()
    bo_s = nc.dram_tensor("bo_s", [P, D // P], F32, kind="ExternalInput").ap()
    vmask = nc.dram_tensor("vmask", [P, NKC], F32, kind="ExternalInput").ap()
    vmask8 = nc.dram_tensor("vmask8", [P, NKC * NH], F16,
                            kind="ExternalInput").ap()
    out_t = nc.dram_tensor("out_t", [D, S], F32, kind="ExternalOutput").ap()

    x_c = x_t.rearrange("(c p) s -> c p s", p=P)
    m_c = mem_t.rearrange("(c p) s -> c p s", p=P)
    wq_c = wq_t.rearrange("(c p) o -> c p o", p=P)
    wk_c = wk_t.rearrange("(c p) o -> c p o", p=P)
    wv_c = wv_t.rearrange("(c p) o -> c p o", p=P)
    wo_c = wo_t.rearrange("(c p) o -> c p o", p=P)

    with tile.TileContext(nc) as tc, ExitStack() as ctx:
        q_pool = ctx.enter_context(tc.tile_pool(name="qt", bufs=1))
        k_pool = ctx.enter_context(tc.tile_pool(name="kt", bufs=1))
        v_pool = ctx.enter_context(tc.tile_pool(name="va", bufs=1))
        a_pool = ctx.enter_context(tc.tile_pool(name="at", bufs=1))
        c_pool = ctx.enter_context(tc.tile_pool(name="cst", bufs=1))
        w_pool = ctx.enter_context(tc.tile_pool(name="wt", bufs=10))
        e_pool = ctx.enter_context(tc.tile_pool(name="es", bufs=16))
        n_pool = ctx.enter_context(tc.tile_pool(name="nrm", bufs=2))
        o_pool = ctx.enter_context(tc.tile_pool(name="ev", bufs=3))
        psum_pool = ctx.enter_context(tc.tile_pool(name="ps", bufs=1, space="PSUM"))
        m_pool = ctx.enter_context(tc.tile_pool(name="mm", bufs=8))
        x_pool = ctx.enter_context(tc.tile_pool(name="xx", bufs=8))

        # ---- constants ----
        bq_sb = c_pool.tile([P, OD // P], F32, tag="bq")
        bk_sb = c_pool.tile([P, OD // P], F32, tag="bk")
        bo_sb = c_pool.tile([P, D // P], F32, tag="bo")
        vm_sb = c_pool.tile([P, NKC], F32, tag="vm")
        vm8_sb = c_pool.tile([P, NKC, NH], F16, tag="vm8")
        nc.sync.dma_start(out=bq_sb[:], in_=bq_s[:])
        nc.sync.dma_start(out=bk_sb[:], in_=bk_s[:])
        nc.sync.dma_start(out=bo_sb[:], in_=bo_s[:])
        nc.sync.dma_start(out=vm_sb[:], in_=vmask[:])
        nc.sync.dma_start(
            out=vm8_sb[:], in_=vmask8.rearrange("p (s h) -> p s h", h=NH))

        # ---- weight DMAs first (small, needed first), then bulk inputs:
        # memory on both queues (K0/V gate on it), then x (Q0 gates on it)
        w0_tiles = {"k": [], "q": [], "v": []}
        for i in range(NDC):
            wt = w_pool.tile([P, P], F16, tag="w", name="wk0", bufs=10)
            nc.sync.dma_start(out=wt[:], in_=wk_c[i, :, 0:P])
            w0_tiles["k"].append(wt)
        for i in range(NDC):
            wt = w_pool.tile([P, P], F16, tag="wq0", name="wq0", bufs=8)
            nc.gpsimd.dma_start(out=wt[:], in_=wq_c[i, :, 0:P])
            w0_tiles["q"].append(wt)
        # m/x as column blocks [128, 512]: block cb holds tokens/queries
        # [cb*512, (cb+1)*512) so compute can start after ~2MB instead of 8.5
        m_cb = [[None] * NDC for _ in range(4)]
        x_cb = [[None] * NDC for _ in range(4)]

        def load_cb(cb):
            for i in range(NDC):
                t = m_pool.tile([P, 512], F16, tag=f"m{cb}", name="mt")
                eng = nc.sync if (i + cb) % 2 == 0 else nc.gpsimd
                eng.dma_start(out=t[:], in_=m_c[i, :, cb * 512:(cb + 1) * 512])
                m_cb[cb][i] = t
            for i in range(NDC):
                t = x_pool.tile([P, 512], F16, tag=f"x{cb}", name="xt")
                eng = nc.sync if (i + cb) % 2 == 1 else nc.gpsimd
                eng.dma_start(out=t[:], in_=x_c[i, :, cb * 512:(cb + 1) * 512])
                x_cb[cb][i] = t
        load_cb(0)
        for i in range(NDC):
            wt = w_pool.tile([P, OD], F16, tag="wv", name="wvt", bufs=8)
            eng = nc.sync if i % 2 == 0 else nc.gpsimd
            eng.dma_start(out=wt[:], in_=wv_c[i])
            w0_tiles["v"].append(wt)
        for cb in (1, 2, 3):
            load_cb(cb)

        # ---- persistent tiles ----
        qT = [q_pool.tile([P, S], F16, tag=f"q{m}", name=f"q{m}")
              for m in range(NMT)]
        # kT packs a head pair: partitions 0:64 = head 2m, 64:128 = head 2m+1
        kT = [k_pool.tile([P, S], F16, tag=f"k{m}", name=f"k{m}")
              for m in range(NMT)]
        v_aug = [v_pool.tile([P, NH, 65], F16, tag=f"v{st}", name=f"v{st}")
                 for st in range(NKC)]
        attn = [a_pool.tile([P, S], F16, tag=f"a{m}", name=f"a{m}")
                for m in range(NMT)]

        # ---- filler stream (projections chopped into single-MM steps) ----
        fills = deque()

        def pump(n):
            for _ in range(n):
                if fills:
                    fills.popleft()()

        def kq_proj_steps(wc, src_cb, dst, bias, m, w_tiles=None,
                          halves=range(4)):
            if w_tiles is None:
                w_tiles = []

                def load_w():
                    for i in range(NDC):
                        wt = w_pool.tile([P, P], F16, tag="w", name="wkq",
                                         bufs=10)
                        nc.sync.dma_start(
                            out=wt[:], in_=wc[i, :, m * P:(m + 1) * P])
                        w_tiles.append(wt)
                steps = [load_w]
            else:
                steps = []
            for half in halves:
                csl = slice(half * 512, (half + 1) * 512)
                ps = []

                def mm(i, ps=ps, half=half):
                    if i == 0:
                        ps.append(psum_pool.tile([P, 512], F32, tag="pj",
                                                 name="pskq", bufs=2))
                    nc.tensor.matmul(
                        ps[0][:], w_tiles[i][:], src_cb[half][i][:],
                        start=(i == 0), stop=(i == NDC - 1))
                for i in range(NDC):
                    steps.append(lambda i=i, mm=mm: mm(i))

                def evac(ps=ps, csl=csl):
                    nc.vector.tensor_scalar_add(
                        dst[:, csl], ps[0][:], bias[:, m:m + 1])
                steps.append(evac)
            return steps

        # V-proj: per-token-chunk projection (preloaded weights)
        wv_tiles = w0_tiles["v"]

        def v_proj(st):
            ps = psum_pool.tile([P, 512], F32, tag="pj", name="psv", bufs=2)
            for i in range(NDC):
                nc.tensor.matmul(
                    ps[:], m_cb[st // 4][i][:, (st % 4) * P:(st % 4 + 1) * P],
                    wv_tiles[i][:],
                    start=(i == 0), stop=(i == NDC - 1))
            nc.vector.tensor_scalar_mul(
                v_aug[st][:, 0:NH, 0:64],
                ps[:].rearrange("p (h d) -> p h d", h=NH),
                vm_sb[:, st:st + 1])
            nc.gpsimd.tensor_copy(
                v_aug[st][:, 0:NH, 64:65], vm8_sb[:, st, :].unsqueeze(2))

        wo_tiles = [[None] * NMT for _ in range(D // P)]

        def load_wo(m):
            for i in range(NMT):
                wt = w_pool.tile([P, P], F16, tag="wo", name="wot", bufs=32)
                nc.sync.dma_start(out=wt[:], in_=wo_c[i, :, m * P:(m + 1) * P])
                wo_tiles[m][i] = wt

        def o_proj_steps(m, jb):
            ps = []

            def mm(i):
                if i == 0:
                    ps.append(psum_pool.tile([P, 512], F32, tag="pj",
                                             name="pso", bufs=2))
                nc.tensor.matmul(
                    ps[0][:], wo_tiles[m][i][:],
                    attn[i][:, jb * 512:(jb + 1) * 512],
                    start=(i == 0), stop=(i == NMT - 1))
            steps = [lambda i=i, mm=mm: mm(i) for i in range(NMT)]

            def evac():
                ev = o_pool.tile([P, 512], F32, tag="ev")
                nc.vector.tensor_scalar_add(ev[:], ps[0][:], bo_sb[:, m:m + 1])
                nc.sync.dma_start(
                    out=out_t[m * P:(m + 1) * P, jb * 512:(jb + 1) * 512],
                    in_=ev[:])
            steps.append(evac)
            return steps

        # Q weights for all head-pairs preloaded (tiny); halves emitted
        # on demand at loop tops
        wq_all = [w0_tiles["q"]]
        for m in (1, 2, 3):
            tiles = []
            for i in range(NDC):
                wt = w_pool.tile([P, P], F16, tag=f"wq{m}", name="wqm", bufs=8)
                nc.sync.dma_start(out=wt[:], in_=wq_c[i, :, m * P:(m + 1) * P])
                tiles.append(wt)
            wq_all.append(tiles)

        # ---- attention pipeline ----
        esbuf = {}
        avbuf = {}

        def emit_A(g, k2):
            qb, mt = divmod(g, 4)
            qsl = slice(qb * 512, (qb + 1) * 512)
            ka, kb = 2 * k2, 2 * k2 + 1
            lg_e = psum_pool.tile([P, 1024], F32, tag="lg", name="lg_e",
                                  bufs=2)
            lg_o = psum_pool.tile([P, 1024], F32, tag="lg", name="lg_o",
                                  bufs=2)
            for half, kc in ((0, ka), (1, kb)):
                nc.tensor.matmul(
                    lg_e[:, half * 512:(half + 1) * 512],
                    kT[mt][0:64, kc * P:(kc + 1) * P],
                    qT[mt][0:64, qsl], start=True, stop=True)
                nc.tensor.matmul(
                    lg_o[:, half * 512:(half + 1) * 512],
                    kT[mt][64:128, kc * P:(kc + 1) * P],
                    qT[mt][64:128, qsl], start=True, stop=True)
            es_e = e_pool.tile([P, 1024], F16, tag="es", bufs=16)
            nc.scalar.activation(es_e[:], lg_e[:], AF.Exp, scale=0.125)
            es_o = e_pool.tile([P, 1024], F16, tag="es", bufs=16)
            nc.scalar.activation(es_o[:], lg_o[:], AF.Exp, scale=0.125)
            esbuf[(g, k2)] = (es_e, es_o)
            if g >= 2:
                pump(3)

        def emit_B(g, k2):
            qb, mt = divmod(g, 4)
            he, ho = 2 * mt, 2 * mt + 1
            if k2 == 0:
                avbuf[g] = (
                    psum_pool.tile([P, 512], F32, tag="av_e", name="av_e"),
                    psum_pool.tile([P, 512], F32, tag="av_o", name="av_o"))
            av_e, av_o = avbuf[g]
            if g == 0:
                for st in ([8 + 2 * k2, 9 + 2 * k2] if k2 < 2 else
                           [10 + k2] if k2 < 6 else []):
                    v_proj(st)
            es_e, es_o = esbuf.pop((g, k2))
            ka, kb = 2 * k2, 2 * k2 + 1
            for half, kc in ((0, ka), (1, kb)):
                va = v_aug[kc][:].rearrange("p h d -> p (h d)")
                nc.tensor.matmul(
                    av_e[0:65, :], va[:, 65 * he:65 * he + 65],
                    es_e[:, half * 512:(half + 1) * 512],
                    start=(k2 == 0 and half == 0),
                    stop=(k2 == NKC // 2 - 1 and half == 1))
            for half, kc in ((0, ka), (1, kb)):
                va = v_aug[kc][:].rearrange("p h d -> p (h d)")
                nc.tensor.matmul(
                    av_o[0:65, :], va[:, 65 * ho:65 * ho + 65],
                    es_o[:, half * 512:(half + 1) * 512],
                    start=(k2 == 0 and half == 0),
                    stop=(k2 == NKC // 2 - 1 and half == 1))
            if g >= 1:
                pump(2)

        def emit_norm(g):
            qb, mt = divmod(g, 4)
            qsl = slice(qb * 512, (qb + 1) * 512)
            av_e, av_o = avbuf.pop(g)
            for ro, av in ((0, av_e), (64, av_o)):
                dn = n_pool.tile([1, 512], F32, tag="dn")
                r0 = n_pool.tile([1, 512], F32, tag="r0")
                bc = n_pool.tile([64, 512], F32, tag="bc")
                nc.vector.tensor_copy(dn[:], av[64:65, :])
                nc.vector.reciprocal_approx_fast(out=r0[:], in_=dn[:])
                nc.gpsimd.partition_broadcast(bc[:], r0[:])
                nc.vector.tensor_mul(
                    attn[mt][ro:ro + 64, qsl], av[0:64, :], bc[:])

        # staged startup: K0 half-by-half interleaved with A(0) so the
        # first exp fires as soon as 2MB (m/x column-block 0) has landed
        k0 = kq_proj_steps(wk_c, m_cb, kT[0], bk_sb, 0, w_tiles=w0_tiles["k"])
        q0 = kq_proj_steps(wq_c, x_cb, qT[0], bq_sb, 0, w_tiles=w0_tiles["q"],
                           halves=[0])
        for step in k0[0:9]:     # K0 half0
            step()
        for step in q0:          # Q0 half0 (all A(0) needs)
            step()
        emit_A(0, 0)
        emit_A(0, 1)
        for step in k0[9:18]:    # K0 half1
            step()
        v_proj(0)
        v_proj(1)
        emit_A(0, 2)
        emit_A(0, 3)
        for step in k0[18:27]:   # K0 half2
            step()
        v_proj(2)
        v_proj(3)
        emit_A(0, 4)
        emit_A(0, 5)
        for step in k0[27:36]:   # K0 half3
            step()
        v_proj(4)
        v_proj(5)
        emit_A(0, 6)
        emit_A(0, 7)
        v_proj(6)
        v_proj(7)

        def emit_k(m):
            for step in kq_proj_steps(wk_c, m_cb, kT[m], bk_sb, m):
                step()

        def emit_qh(mt, qb):
            for step in kq_proj_steps(wq_c, x_cb, qT[mt], bq_sb, mt,
                                      w_tiles=wq_all[mt], halves=[qb]):
                step()

        # depth-2 pipeline, qb-major g = (qb, mt): A(g) runs two phases
        # ahead of B(g) (16-deep es ring = exactly two phases), so the exp
        # stream never waits on AV; K2/K3/Q-halves are emitted at loop tops
        # just-in-time for A(g+2); O-proj streams per finished query column
        emit_k(1)
        emit_qh(1, 0)
        for k2 in range(NKC // 2):
            emit_A(1, k2)
        qh_done = {(0, 0), (1, 0)}
        for g in range(NG):
            if g + 2 < NG:
                qb2, mt2 = divmod(g + 2, 4)
                if mt2 == 2 and qb2 == 0:
                    emit_k(2)
                if mt2 == 3 and qb2 == 0:
                    emit_k(3)
                if (mt2, qb2) not in qh_done:
                    qh_done.add((mt2, qb2))
                    emit_qh(mt2, qb2)
            for k2 in range(NKC // 2):
                emit_B(g, k2)
                if g + 2 < NG:
                    emit_A(g + 2, k2)
            emit_norm(g)
            qb, mt = divmod(g, 4)
            if mt == NMT - 1 and qb < 3:
                for m in range(D // P):
                    if qb == 0:
                        fills.append(lambda m=m: load_wo(m))
                    fills.extend(o_proj_steps(m, qb))

        # ---- drain fills, then O-proj for the last column block ----
        while fills:
            fills.popleft()()
        for m in range(D // P):
            for step in o_proj_steps(m, 3):
                step()

    nc.compile()
    return nc


def _prep_inputs(x, memory, mask, wq, bq, wk, bk, wv, bv, wo, bo):
    f = np.float32
    h = np.float16
    wqT = np.ascontiguousarray(wq.T, dtype=f)
    wkT = np.ascontiguousarray(wk.T, dtype=f)
    wvT = np.ascontiguousarray(wv.T, dtype=f)
    woT = np.ascontiguousarray(wo.T, dtype=f)
    bo_eff = (bo.astype(f) + wo.astype(f) @ bv.astype(f))
    zeros_bo = np.zeros_like(bo_eff)
    in_maps = []
    for c in range(NCORES):
        b, g = divmod(c, 2)
        sl = slice(g * OD, (g + 1) * OD)
        bo_c = bo_eff if g == 0 else zeros_bo
        vm = np.where(mask[b], np.float32(0.0), np.float32(1.0)).astype(f)
        vm_s = np.ascontiguousarray(vm.reshape(NKC, P).T)      # [P, NKC]
        vm8 = np.repeat(vm_s.astype(h)[:, :, None], NH, axis=2)  # [P,NKC,NH]
        in_maps.append({
            "x_t": np.ascontiguousarray(x[b].T, dtype=h),
            "mem_t": np.ascontiguousarray(memory[b].T, dtype=h),
            "wq_t": np.ascontiguousarray(wqT[:, sl]).astype(h),
            "wk_t": np.ascontiguousarray(wkT[:, sl]).astype(h),
            "wv_t": np.ascontiguousarray(wvT[:, sl]).astype(h),
            "wo_t": np.ascontiguousarray(woT[sl, :]).astype(h),
            "bq_s": np.ascontiguousarray(bq[sl].astype(f).reshape(OD // P, P).T),
            "bk_s": np.ascontiguousarray(bk[sl].astype(f).reshape(OD // P, P).T),
            "bo_s": np.ascontiguousarray(bo_c.reshape(D // P, P).T),
            "vmask": vm_s,
            "vmask8": np.ascontiguousarray(vm8.reshape(P, NKC * NH)),
        })
    return in_maps


def kernel(x, memory, mask, wq, bq, wk, bk, wv, bv, wo, bo, **run_kwargs):
    x = np.asarray(x, dtype=np.float32)
    memory = np.asarray(memory, dtype=np.float32)
    mask = np.asarray(mask)
    if "nc" not in _cache:
        _cache["nc"] = _build()
    nc = _cache["nc"]
    in_maps = _prep_inputs(x, memory, mask, wq, bq, wk, bk, wv, bv, wo, bo)
    res = run_bass_kernel_spmd(nc, in_maps, list(range(NCORES)), **run_kwargs)
    out = np.empty((B, S, D), dtype=np.float32)
    for b in range(B):
        part = res.results[2 * b]["out_t"] + res.results[2 * b + 1]["out_t"]
        out[b] = part.T
    if run_kwargs:
        _cache["last_results"] = res
    return out
